# revision 28
# baseline (speedup 1.0000x reference)
"""Trainium2 Bass kernel v3 for MeshConv-style GNN message passing.

v3 over v2: one z1 pass per window (multi-bank affine PSUM AP), newton
rsqrt batched per 8-window superblock, host-precomputed b2*ind tiles,
engine-rebalanced elementwise chain (DVE/ACT/gpsimd).
"""

import sys

if "/opt/trn_rl_repo" not in sys.path:
    sys.path.insert(0, "/opt/trn_rl_repo")

import numpy as np

N_NODES = 100000
IN_DIM = 128
OUT_DIM = 128
EDGE_DIM = 4
N_GROUPS = 8
GSIZE = 16
EPS = 1e-5

N_CORES = 8
NPC = N_NODES // N_CORES
WIN = 123
TE = 128
SB = 8  # windows per superblock (newton batching)

LAST_EXEC_NS = None
LAST_RESULTS = None
SIM_SAFE_SILU = False
SIM_MODE = False


def _shard(edge_index, edge_attr):
    src = np.ascontiguousarray(edge_index[0]).astype(np.int64)
    dst = np.ascontiguousarray(edge_index[1]).astype(np.int64)
    E = src.shape[0]
    ea = np.ascontiguousarray(edge_attr).astype(np.float32)

    order = np.argsort(dst, kind="stable")
    src = src[order]
    dst = dst[order]
    ea = ea[order]

    core = np.minimum(dst // NPC, N_CORES - 1)
    local = dst - core * NPC
    win = local // WIN
    nwin = (NPC + WIN - 1) // WIN

    cw = core * nwin + win
    counts = np.bincount(cw, minlength=N_CORES * nwin).reshape(N_CORES, nwin)
    T_ws = np.maximum(1, (counts.max(axis=0) + TE - 1) // TE).astype(np.int64)
    total_tiles = int(T_ws.sum())
    cap = total_tiles * TE

    woff = np.zeros(nwin, dtype=np.int64)
    woff[1:] = np.cumsum(T_ws)[:-1] * TE
    cw_starts = np.zeros(N_CORES * nwin, dtype=np.int64)
    cw_starts[1:] = np.cumsum(counts.reshape(-1))[:-1]
    pos_in_cw = np.arange(E, dtype=np.int64) - cw_starts[cw]
    slot = woff[win] + pos_in_cw

    node_counts = np.bincount(dst, minlength=N_NODES).astype(np.float32)

    shards = []
    for c in range(N_CORES):
        m = core == c
        sl = slot[m]
        srcc = np.zeros((cap,), dtype=np.int64)
        srcc[sl] = src[m]
        dstc = np.zeros((cap,), dtype=np.int64)
        dstc[sl] = dst[m]
        dloc = np.full((cap,), -1, dtype=np.int64)
        dloc[sl] = local[m] - win[m] * WIN
        eac = np.zeros((cap, EDGE_DIM), dtype=np.float32)
        eac[sl] = ea[m]
        shards.append((srcc, dstc, dloc, eac))
    return T_ws, nwin, cap, node_counts, shards


def _build_program(T_ws, nwin):
    import concourse.bacc as bacc
    from concourse import mybir
    from concourse.tile import TileContext

    f32 = mybir.dt.float32
    f16 = mybir.dt.float16
    i32 = mybir.dt.int32
    AF = mybir.ActivationFunctionType
    OP = mybir.AluOpType

    total_tiles = int(sum(T_ws))
    D1 = 136
    nb_max = max((int(t) + 2) // 3 for t in T_ws)
    assert nb_max <= 3, f"window too full: {nb_max}"

    if SIM_MODE:
        nc = bacc.Bacc("TRN2", target_bir_lowering=False, debug=True)
    else:
        nc = bacc.Bacc()
    f8 = mybir.dt.float8e4
    xsT_d = nc.dram_tensor("xsT", [TE, total_tiles * TE], f16, kind="ExternalInput")
    xdT_d = nc.dram_tensor("xdT", [TE, total_tiles * TE], f16, kind="ExternalInput")
    eaT_d = nc.dram_tensor("eaT", [5, total_tiles * TE], f16, kind="ExternalInput")
    st_d = nc.dram_tensor("st", [TE, total_tiles * WIN], f8, kind="ExternalInput")
    w1dg_d = nc.dram_tensor("w1dg", [128, D1], f16, kind="ExternalInput")
    w1sg_d = nc.dram_tensor("w1sg", [128, D1], f16, kind="ExternalInput")
    w1eg_d = nc.dram_tensor("w1eg", [5, D1], f16, kind="ExternalInput")
    w2_d = nc.dram_tensor("w2", [128, 128], f16, kind="ExternalInput")
    b2i_d = nc.dram_tensor("b2i", [nwin * WIN, 128], f16, kind="ExternalInput")
    inv_d = nc.dram_tensor("invc", [WIN, nwin], f32, kind="ExternalInput")
    id_d = nc.dram_tensor("ident", [128, 128], f16, kind="ExternalInput")
    out_d = nc.dram_tensor("out", [nwin * WIN, OUT_DIM], f32, kind="ExternalOutput")

    # superblock window ranges and v-offsets
    sblocks = []
    for s0 in range(0, nwin, SB):
        ws = list(range(s0, min(s0 + SB, nwin)))
        voff = [0]
        for w in ws:
            voff.append(voff[-1] + 8 * int(T_ws[w]))
        sblocks.append((ws, voff))

    with TileContext(nc) as tc:
        with (
            tc.tile_pool(name="const", bufs=1) as cp,
            tc.tile_pool(name="sb", bufs=3) as sp,
            tc.tile_pool(name="stp", bufs=6) as stp,
            tc.tile_pool(name="zz", bufs=2 * SB + 2) as zz,
            tc.tile_pool(name="hh", bufs=3) as hh,
            tc.tile_pool(name="wp", bufs=3) as wp,
            tc.tile_pool(name="vv", bufs=2) as vv,
            tc.tile_pool(name="ph", bufs=2, space="PSUM") as ph,
            tc.tile_pool(name="pu", bufs=1, space="PSUM") as pu,
            tc.tile_pool(name="pf", bufs=1, space="PSUM") as pf,
        ):
            def cload(dram, shape, tag, dt=f16):
                t = cp.tile(shape, dt, tag=tag)
                nc.sync.dma_start(out=t[:], in_=dram[:])
                return t

            W1DG = cload(w1dg_d, [128, D1], "c_w1dg")
            W1SG = cload(w1sg_d, [128, D1], "c_w1sg")
            W1EG = cload(w1eg_d, [5, D1], "c_w1eg")
            W2 = cload(w2_d, [128, 128], "c_w2")
            INV = cload(inv_d, [WIN, nwin], "c_inv", f32)
            IDENT = cload(id_d, [128, 128], "c_id")

            gts = {}
            gt = 0
            for w in range(nwin):
                gts[w] = gt
                gt += int(T_ws[w])

            def run_phase2(state):
                ws, voff, z_tiles, zq_tiles, st_tiles, inv16 = state
                for wi, w in enumerate(ws):
                    Tw = int(T_ws[w])
                    z_all = z_tiles[w]
                    zq = zq_tiles[w]
                    st_w = st_tiles[w]

                    nc.gpsimd.tensor_tensor(
                        out=zq[:].rearrange("p (s c) -> p s c", c=GSIZE),
                        in0=z_all[:, 0:Tw * TE].rearrange(
                            "p (s c) -> p s c", c=GSIZE),
                        in1=inv16[:, voff[wi]:voff[wi + 1], None].to_broadcast(
                            [128, 8 * Tw, GSIZE]),
                        op=OP.mult)
                    hs = hh.tile([128, Tw * TE], f16, tag="hs")
                    if SIM_SAFE_SILU:
                        sg = hh.tile([128, Tw * TE], f16, tag="sg")
                        nc.scalar.activation(out=sg[:], in_=zq[:],
                                             func=AF.Sigmoid)
                        nc.vector.tensor_tensor(out=hs[:], in0=zq[:],
                                                in1=sg[:], op=OP.mult)
                    else:
                        nc.scalar.activation(out=hs[:], in_=zq[:],
                                             func=AF.Silu)

                    u_pf = pu.tile([128, 512], f32, tag="up")
                    u_p = u_pf[0:WIN, 0:128]
                    for t in range(Tw):
                        nc.tensor.matmul(
                            u_p, lhsT=st_w[:, t * WIN:(t + 1) * WIN],
                            rhs=hs[:, t * TE:(t + 1) * TE],
                            start=(t == 0), stop=(t == Tw - 1))

                    b2i_w = wp.tile([WIN, 128], f16, tag="b2i")
                    nc.sync.dma_start(
                        out=b2i_w[:],
                        in_=b2i_d[w * WIN:(w + 1) * WIN, :])
                    u16 = wp.tile([WIN, 128], f16, tag="u16")
                    nc.vector.tensor_scalar_mul(out=u16[:], in0=u_p,
                                                scalar1=INV[:, w:w + 1])
                    fin_p = pf.tile([128, 512], f32, tag="fin")
                    ut_p = fin_p[:, 0:64].bitcast(f16)[:, 0:WIN]
                    nc.tensor.transpose(ut_p, u16[:], IDENT[0:WIN, 0:WIN])
                    ut16 = wp.tile([128, WIN], f16, tag="ut16")
                    nc.scalar.copy(out=ut16[:], in_=ut_p)
                    o_p = fin_p[0:WIN, 128:256]
                    nc.tensor.matmul(o_p, lhsT=ut16[:], rhs=W2[:],
                                     start=True, stop=True)
                    o_s = wp.tile([WIN, 128], f32, tag="os")
                    nc.vector.tensor_tensor(out=o_s[:], in0=o_p, in1=b2i_w[:],
                                            op=OP.add)
                    nc.sync.dma_start(out=out_d[w * WIN:(w + 1) * WIN, :],
                                      in_=o_s[:])

            pending = None
            for ws, voff in sblocks:
                VW = voff[-1]
                v_sb = vv.tile([128, VW], f32, tag="vsb")
                z_tiles = {}
                zq_tiles = {}
                st_tiles = {}

                # ---- phase 1: MM1 + stats per window ----
                # big input streams batched one DMA per 4-window group
                for wi, w in enumerate(ws):
                    Tw = int(T_ws[w])
                    nb = (Tw + 2) // 3
                    TP = nb * 3
                    gt = gts[w]

                    if wi % 4 == 0:
                        gws = ws[wi:wi + 4]
                        g0 = gts[gws[0]]
                        gT = sum(int(T_ws[x]) for x in gws)
                        xsT_g = sp.tile([TE, gT * TE], f16, tag="xsT")
                        nc.sync.dma_start(
                            out=xsT_g[:],
                            in_=xsT_d[:, g0 * TE:(g0 + gT) * TE])
                        xdT_g = sp.tile([TE, gT * TE], f16, tag="xdT")
                        nc.sync.dma_start(
                            out=xdT_g[:],
                            in_=xdT_d[:, g0 * TE:(g0 + gT) * TE])
                        st_g = stp.tile([TE, gT * WIN], f8, tag="st")
                        nc.sync.dma_start(
                            out=st_g[:],
                            in_=st_d[:, g0 * WIN:(g0 + gT) * WIN])
                    goff = gt - g0
                    xsT_w = xsT_g[:, goff * TE:(goff + Tw) * TE]
                    xdT_w = xdT_g[:, goff * TE:(goff + Tw) * TE]
                    eaT_w = sp.tile([5, Tw * TE], f16, tag="eaT")
                    nc.sync.dma_start(
                        out=eaT_w[:], in_=eaT_d[:, gt * TE:(gt + Tw) * TE])
                    st_tiles[w] = st_g[:, goff * WIN:(goff + Tw) * WIN]

                    h_p = ph.tile([128, nb * 512], f32, tag="h")
                    hv = h_p[:].rearrange("p (b c) -> p b c", c=512)
                    for t in range(Tw):
                        b, k = t // 3, t % 3
                        osl = hv[:, b, k * D1:(k + 1) * D1]
                        nc.tensor.matmul(
                            osl, lhsT=xdT_w[:, t * TE:(t + 1) * TE],
                            rhs=W1DG[:], start=True, stop=False)
                        nc.tensor.matmul(
                            osl, lhsT=xsT_w[:, t * TE:(t + 1) * TE],
                            rhs=W1SG[:], start=False, stop=False)
                        nc.tensor.matmul(
                            osl, lhsT=eaT_w[:, t * TE:(t + 1) * TE],
                            rhs=W1EG[:], start=False, stop=True)

                    if SIM_MODE:
                        # pad tile slots are never consumed; CoreSim still
                        # requires initialized reads
                        for t in range(Tw, TP):
                            b, k = t // 3, t % 3
                            nc.vector.memset(
                                hv[:, b, k * D1:(k + 1) * D1], 0.0)
                    # h viewed [128, nb, 3, 136] (banks hold 3*136 + 104 pad)
                    hv4 = hv[:, :, 0:3 * D1].rearrange(
                        "p b (k c) -> p b k c", c=D1)
                    # mu: [128, nb, 3, 8] from h cols 128:136 (ACT copy)
                    mu_sb = wp.tile([128, nb_max * 24], f32, tag="mu")
                    muv = mu_sb[:, 0:TP * 8].rearrange(
                        "p (b k g) -> p b k g", k=3, g=8)
                    nc.scalar.copy(out=muv, in_=hv4[:, :, :, 128:136])

                    z_all = zz.tile([128, TP * TE], f16, tag="zall")
                    z_tiles[w] = z_all
                    # z1 = h - mu, per bank (ISA: STT inputs must lower to 3D)
                    zb = z_all[:].rearrange(
                        "p (b k g c) -> p b k g c", k=3, g=8, c=GSIZE)
                    for b in range(nb):
                        nc.vector.scalar_tensor_tensor(
                            out=zb[:, b],
                            in0=muv[:, b][:, :, :, None].to_broadcast(
                                [128, 3, 8, GSIZE]),
                            scalar=-1.0, op0=OP.mult, op1=OP.add,
                            in1=hv4[:, b, :, 0:128].rearrange(
                                "p k (g c) -> p k g c", c=GSIZE),
                        )
                    zq = zz.tile([128, Tw * TE], f16, tag="zsq")
                    zq_tiles[w] = zq
                    nc.scalar.activation(out=zq[:], in_=z_all[:, 0:Tw * TE],
                                         func=AF.Square)
                    # group sums: halving adds  16->8->4->2->1
                    zv = zq[:].rearrange("p (s g c) -> p s g c", g=8, c=GSIZE)
                    r8 = hh.tile([128, Tw * 64], f16, tag="r8")
                    r8v = r8[:].rearrange("p (s g c) -> p s g c", g=8, c=8)
                    nc.gpsimd.tensor_tensor(
                        out=r8v, in0=zv[:, :, :, 0:8], in1=zv[:, :, :, 8:16],
                        op=OP.add)
                    r4 = hh.tile([128, Tw * 32], f16, tag="r4")
                    r4v = r4[:].rearrange("p (s g c) -> p s g c", g=8, c=4)
                    nc.gpsimd.tensor_tensor(
                        out=r4v, in0=r8v[:, :, :, 0:4], in1=r8v[:, :, :, 4:8],
                        op=OP.add)
                    r2 = hh.tile([128, Tw * 16], f16, tag="r2")
                    r2v = r2[:].rearrange("p (s g c) -> p s g c", g=8, c=2)
                    nc.vector.tensor_tensor(
                        out=r2v, in0=r4v[:, :, :, 0:2], in1=r4v[:, :, :, 2:4],
                        op=OP.add)
                    nc.vector.tensor_tensor(
                        out=v_sb[:, voff[wi]:voff[wi + 1]].rearrange(
                            "p (s c) -> p s c", c=1),
                        in0=r2v[:, :, :, 0:1].rearrange("p s g c -> p (s g) c"),
                        in1=r2v[:, :, :, 1:2].rearrange("p s g c -> p (s g) c"),
                        op=OP.add)

                # ---- newton rsqrt batched over the superblock ----
                # vh = var/2 = v/32 + eps/2 ; quake seed vs vh needs the
                # magic shifted by 2^22 (vh = var/2 halves the exponent arg)
                vh = vv.tile([128, VW], f32, tag="vh")
                nc.vector.tensor_scalar(out=vh[:], in0=v_sb[:],
                                        scalar1=1.0 / (2 * GSIZE),
                                        scalar2=EPS / 2,
                                        op0=OP.mult, op1=OP.add)
                y = vv.tile([128, VW], f32, tag="y")
                nc.vector.tensor_scalar(
                    out=y[:].bitcast(i32), in0=vh[:].bitcast(i32), scalar1=1,
                    scalar2=None, op0=OP.logical_shift_right)
                nc.vector.tensor_scalar(
                    out=y[:].bitcast(i32), in0=y[:].bitcast(i32), scalar1=-1,
                    scalar2=0x5EF759DF, op0=OP.mult, op1=OP.add)
                a = vv.tile([128, VW], f32, tag="nta")
                nc.vector.tensor_tensor(out=a[:], in0=y[:], in1=y[:],
                                        op=OP.mult)
                nc.vector.tensor_tensor(out=a[:], in0=a[:], in1=vh[:],
                                        op=OP.mult)
                nc.vector.tensor_scalar(out=a[:], in0=a[:], scalar1=-1.0,
                                        scalar2=1.5, op0=OP.mult, op1=OP.add)
                inv16 = vv.tile([128, VW], f16, tag="inv16")
                nc.vector.tensor_tensor(out=inv16[:], in0=y[:], in1=a[:],
                                        op=OP.mult)

                state = (ws, voff, z_tiles, zq_tiles, st_tiles, inv16)
                if pending is not None:
                    run_phase2(pending)
                pending = state

            run_phase2(pending)

    nc.compile()
    return nc


def _prepare(x, edge_index, edge_attr, W1, b1, gn_gamma, gn_beta, W2, b2):
    x = np.ascontiguousarray(np.asarray(x, dtype=np.float32))
    W1 = np.asarray(W1, dtype=np.float32)
    b1 = np.asarray(b1, dtype=np.float32)
    W2 = np.asarray(W2, dtype=np.float32)
    b2 = np.asarray(b2, dtype=np.float32)
    gn_gamma = np.asarray(gn_gamma, dtype=np.float32)
    gn_beta = np.asarray(gn_beta, dtype=np.float32)

    T_ws, nwin, cap, node_counts, shards = _shard(np.asarray(edge_index),
                                                  edge_attr)
    nc = _build_program(T_ws, nwin)
    total_tiles = cap // TE

    G = np.zeros((OUT_DIM, N_GROUPS), dtype=np.float32)
    for g in range(N_GROUPS):
        G[g * GSIZE:(g + 1) * GSIZE, g] = 1.0 / GSIZE

    trivial = bool(np.all(gn_gamma == 1.0) and np.all(gn_beta == 0.0))
    assert trivial, "nontrivial GN affine not supported"

    W1d, W1s, W1e = W1[0:128], W1[128:256], W1[256:260]
    w1dg = np.concatenate([W1d, W1d @ G], axis=1).astype(np.float16)
    w1sg = np.concatenate([W1s, W1s @ G], axis=1).astype(np.float16)
    w1e_aug = np.concatenate([W1e, b1[None, :]], axis=0)
    w1eg = np.concatenate([w1e_aug, w1e_aug @ G], axis=1).astype(np.float16)

    x16 = x.astype(np.float16)
    ident = np.eye(128, dtype=np.float16)

    shared = {
        "w1dg": np.ascontiguousarray(w1dg),
        "w1sg": np.ascontiguousarray(w1sg),
        "w1eg": np.ascontiguousarray(w1eg),
        "w2": np.ascontiguousarray(W2).astype(np.float16),
        "ident": ident,
    }

    in_maps = []
    for c in range(N_CORES):
        srcc, dstc, dloc, eac = shards[c]
        xsT = np.ascontiguousarray(x16[srcc].T)
        xdT = np.ascontiguousarray(x16[dstc].T)
        eaT = np.empty((5, cap), dtype=np.float16)
        eaT[0:EDGE_DIM] = eac.T.astype(np.float16)
        eaT[EDGE_DIM] = 1.0
        import ml_dtypes
        st = np.zeros((TE, total_tiles * WIN), dtype=ml_dtypes.float8_e4m3)
        dl = dloc.reshape(total_tiles, TE)
        tt_idx, e_idx = np.nonzero(dl >= 0)
        st[e_idx, tt_idx * WIN + dl[tt_idx, e_idx]] = 1.0
        nrows = min(NPC, N_NODES - c * NPC)
        cnt = np.zeros((nwin * WIN,), dtype=np.float32)
        cnt[:nrows] = node_counts[c * NPC:c * NPC + nrows]
        invc = (1.0 / np.maximum(cnt, 1.0)).reshape(nwin, WIN).T
        b2i = ((cnt > 0).astype(np.float32)[:, None]
               * b2[None, :].astype(np.float32)).astype(np.float16)
        in_maps.append(dict(
            shared, xsT=xsT, xdT=xdT, eaT=eaT, st=st,
            invc=np.ascontiguousarray(invc),
            b2i=np.ascontiguousarray(b2i)))
    return nc, in_maps, nwin


def kernel(x, edge_index, edge_attr, W1, b1, gn_gamma, gn_beta, W2, b2):
    global LAST_EXEC_NS, LAST_RESULTS
    import os
    from concourse.bass_utils import run_bass_kernel_spmd

    nc, in_maps, nwin = _prepare(x, edge_index, edge_attr, W1, b1,
                                 gn_gamma, gn_beta, W2, b2)
    trace = bool(os.environ.get("BASS_TRACE"))
    res = run_bass_kernel_spmd(nc, in_maps, core_ids=list(range(N_CORES)),
                               trace=trace)
    LAST_EXEC_NS = res.exec_time_ns
    LAST_RESULTS = res

    out = np.empty((N_NODES, OUT_DIM), dtype=np.float32)
    for c in range(N_CORES):
        out[c * NPC:(c + 1) * NPC] = res.results[c]["out"][:NPC]
    return out


# revision 34
# speedup vs baseline: 1.0578x; 1.0578x over previous
"""Trainium2 Bass kernel v3 for MeshConv-style GNN message passing.

v3 over v2: one z1 pass per window (multi-bank affine PSUM AP), newton
rsqrt batched per 8-window superblock, host-precomputed b2*ind tiles,
engine-rebalanced elementwise chain (DVE/ACT/gpsimd).
"""

import sys

if "/opt/trn_rl_repo" not in sys.path:
    sys.path.insert(0, "/opt/trn_rl_repo")

import numpy as np

N_NODES = 100000
IN_DIM = 128
OUT_DIM = 128
EDGE_DIM = 4
N_GROUPS = 8
GSIZE = 16
EPS = 1e-5

N_CORES = 8
NPC = N_NODES // N_CORES
WIN = 123
TE = 128
SB = 8  # windows per superblock (newton batching)

LAST_EXEC_NS = None
LAST_RESULTS = None
SIM_SAFE_SILU = False
SIM_MODE = False


def _shard(edge_index, edge_attr):
    src = np.ascontiguousarray(edge_index[0]).astype(np.int64)
    dst = np.ascontiguousarray(edge_index[1]).astype(np.int64)
    E = src.shape[0]
    ea = np.ascontiguousarray(edge_attr).astype(np.float32)

    order = np.argsort(dst, kind="stable")
    src = src[order]
    dst = dst[order]
    ea = ea[order]

    core = np.minimum(dst // NPC, N_CORES - 1)
    local = dst - core * NPC
    win = local // WIN
    nwin = (NPC + WIN - 1) // WIN

    cw = core * nwin + win
    counts = np.bincount(cw, minlength=N_CORES * nwin).reshape(N_CORES, nwin)
    T_ws = np.maximum(1, (counts.max(axis=0) + TE - 1) // TE).astype(np.int64)
    total_tiles = int(T_ws.sum())
    cap = total_tiles * TE

    woff = np.zeros(nwin, dtype=np.int64)
    woff[1:] = np.cumsum(T_ws)[:-1] * TE
    cw_starts = np.zeros(N_CORES * nwin, dtype=np.int64)
    cw_starts[1:] = np.cumsum(counts.reshape(-1))[:-1]
    pos_in_cw = np.arange(E, dtype=np.int64) - cw_starts[cw]
    slot = woff[win] + pos_in_cw

    node_counts = np.bincount(dst, minlength=N_NODES).astype(np.float32)

    shards = []
    for c in range(N_CORES):
        m = core == c
        sl = slot[m]
        srcc = np.zeros((cap,), dtype=np.int64)
        srcc[sl] = src[m]
        dstc = np.zeros((cap,), dtype=np.int64)
        dstc[sl] = dst[m]
        dloc = np.full((cap,), -1, dtype=np.int64)
        dloc[sl] = local[m] - win[m] * WIN
        eac = np.zeros((cap, EDGE_DIM), dtype=np.float32)
        eac[sl] = ea[m]
        shards.append((srcc, dstc, dloc, eac))
    return T_ws, nwin, cap, node_counts, shards


def _build_program(T_ws, nwin):
    import concourse.bacc as bacc
    from concourse import mybir
    from concourse.tile import TileContext

    f32 = mybir.dt.float32
    f16 = mybir.dt.float16
    i32 = mybir.dt.int32
    AF = mybir.ActivationFunctionType
    OP = mybir.AluOpType

    total_tiles = int(sum(T_ws))
    D1 = 136
    nb_max = max((int(t) + 2) // 3 for t in T_ws)
    assert nb_max <= 3, f"window too full: {nb_max}"

    if SIM_MODE:
        nc = bacc.Bacc("TRN2", target_bir_lowering=False, debug=True)
    else:
        nc = bacc.Bacc()
    f8 = mybir.dt.float8e4
    xsT_d = nc.dram_tensor("xsT", [TE, total_tiles * TE], f16, kind="ExternalInput")
    xdT_d = nc.dram_tensor("xdT", [TE, total_tiles * TE], f16, kind="ExternalInput")
    eaT_d = nc.dram_tensor("eaT", [5, total_tiles * TE], f16, kind="ExternalInput")
    st_d = nc.dram_tensor("st", [TE, total_tiles * WIN], f8, kind="ExternalInput")
    w1dg_d = nc.dram_tensor("w1dg", [128, D1], f16, kind="ExternalInput")
    w1sg_d = nc.dram_tensor("w1sg", [128, D1], f16, kind="ExternalInput")
    w1eg_d = nc.dram_tensor("w1eg", [5, D1], f16, kind="ExternalInput")
    w2_d = nc.dram_tensor("w2", [128, 128], f16, kind="ExternalInput")
    b2i_d = nc.dram_tensor("b2i", [nwin * WIN, 128], f16, kind="ExternalInput")
    inv_d = nc.dram_tensor("invc", [WIN, nwin], f32, kind="ExternalInput")
    id_d = nc.dram_tensor("ident", [128, 128], f16, kind="ExternalInput")
    out_d = nc.dram_tensor("out", [nwin * WIN, OUT_DIM], f32, kind="ExternalOutput")

    # superblock window ranges and v-offsets
    sblocks = []
    for s0 in range(0, nwin, SB):
        ws = list(range(s0, min(s0 + SB, nwin)))
        voff = [0]
        for w in ws:
            voff.append(voff[-1] + 8 * int(T_ws[w]))
        sblocks.append((ws, voff))

    with TileContext(nc) as tc:
        with (
            tc.tile_pool(name="const", bufs=1) as cp,
            tc.tile_pool(name="sb", bufs=3) as sp,
            tc.tile_pool(name="stp", bufs=6) as stp,
            tc.tile_pool(name="zz", bufs=2 * SB + 2) as zz,
            tc.tile_pool(name="hh", bufs=3) as hh,
            tc.tile_pool(name="wp", bufs=3) as wp,
            tc.tile_pool(name="vv", bufs=2) as vv,
            tc.tile_pool(name="ph", bufs=2, space="PSUM") as ph,
            tc.tile_pool(name="pu", bufs=2, space="PSUM") as pu,
        ):
            def cload(dram, shape, tag, dt=f16):
                t = cp.tile(shape, dt, tag=tag)
                nc.sync.dma_start(out=t[:], in_=dram[:])
                return t

            W1DG = cload(w1dg_d, [128, D1], "c_w1dg")
            W1SG = cload(w1sg_d, [128, D1], "c_w1sg")
            W1EG = cload(w1eg_d, [5, D1], "c_w1eg")
            W2 = cload(w2_d, [128, 128], "c_w2")
            INV = cload(inv_d, [WIN, nwin], "c_inv", f32)
            IDENT = cload(id_d, [128, 128], "c_id")

            gts = {}
            gt = 0
            for w in range(nwin):
                gts[w] = gt
                gt += int(T_ws[w])

            def run_phase2(state):
                ws, voff, z_tiles, zq_tiles, st_tiles, inv16 = state
                for wi, w in enumerate(ws):
                    Tw = int(T_ws[w])
                    z_all = z_tiles[w]
                    zq = zq_tiles[w]
                    st_w = st_tiles[w]

                    nc.gpsimd.tensor_tensor(
                        out=zq[:].rearrange("p (s c) -> p s c", c=GSIZE),
                        in0=z_all[:, 0:Tw * TE].rearrange(
                            "p (s c) -> p s c", c=GSIZE),
                        in1=inv16[:, voff[wi]:voff[wi + 1], None].to_broadcast(
                            [128, 8 * Tw, GSIZE]),
                        op=OP.mult)
                    hs = hh.tile([128, Tw * TE], f16, tag="hs")
                    if SIM_SAFE_SILU:
                        sg = hh.tile([128, Tw * TE], f16, tag="sg")
                        nc.scalar.activation(out=sg[:], in_=zq[:],
                                             func=AF.Sigmoid)
                        nc.vector.tensor_tensor(out=hs[:], in0=zq[:],
                                                in1=sg[:], op=OP.mult)
                    else:
                        nc.scalar.activation(out=hs[:], in_=zq[:],
                                             func=AF.Silu)

                    u_pf = pu.tile([128, 512], f32, tag="up")
                    u_p = u_pf[0:WIN, 0:128]
                    for t in range(Tw):
                        nc.tensor.matmul(
                            u_p, lhsT=st_w[:, t * WIN:(t + 1) * WIN],
                            rhs=hs[:, t * TE:(t + 1) * TE],
                            start=(t == 0), stop=(t == Tw - 1))

                    b2i_w = wp.tile([WIN, 128], f16, tag="b2i")
                    nc.sync.dma_start(
                        out=b2i_w[:],
                        in_=b2i_d[w * WIN:(w + 1) * WIN, :])
                    u16 = wp.tile([WIN, 128], f16, tag="u16")
                    nc.vector.tensor_scalar_mul(out=u16[:], in0=u_p,
                                                scalar1=INV[:, w:w + 1])
                    # finalize reuses this window's u bank (u is dead after
                    # the u16 read); strictly sequential group reuse
                    ut_p = u_pf[:, 128:192].bitcast(f16)[:, 0:WIN]
                    nc.tensor.transpose(ut_p, u16[:], IDENT[0:WIN, 0:WIN])
                    ut16 = wp.tile([128, WIN], f16, tag="ut16")
                    nc.scalar.copy(out=ut16[:], in_=ut_p)
                    o_p = u_pf[0:WIN, 192:320]
                    nc.tensor.matmul(o_p, lhsT=ut16[:], rhs=W2[:],
                                     start=True, stop=True)
                    o_s = wp.tile([WIN, 128], f32, tag="os")
                    nc.vector.tensor_tensor(out=o_s[:], in0=o_p, in1=b2i_w[:],
                                            op=OP.add)
                    nc.sync.dma_start(out=out_d[w * WIN:(w + 1) * WIN, :],
                                      in_=o_s[:])

            pending = None
            for ws, voff in sblocks:
                VW = voff[-1]
                v_sb = vv.tile([128, VW], f32, tag="vsb")
                z_tiles = {}
                zq_tiles = {}
                st_tiles = {}

                # ---- phase 1: MM1 + stats per window ----
                # big input streams batched one DMA per 4-window group
                for wi, w in enumerate(ws):
                    Tw = int(T_ws[w])
                    nb = (Tw + 2) // 3
                    TP = nb * 3
                    gt = gts[w]

                    if wi % 4 == 0:
                        gws = ws[wi:wi + 4]
                        g0 = gts[gws[0]]
                        gT = sum(int(T_ws[x]) for x in gws)
                        xsT_g = sp.tile([TE, gT * TE], f16, tag="xsT")
                        nc.sync.dma_start(
                            out=xsT_g[:],
                            in_=xsT_d[:, g0 * TE:(g0 + gT) * TE])
                        xdT_g = sp.tile([TE, gT * TE], f16, tag="xdT")
                        nc.sync.dma_start(
                            out=xdT_g[:],
                            in_=xdT_d[:, g0 * TE:(g0 + gT) * TE])
                        st_g = stp.tile([TE, gT * WIN], f8, tag="st")
                        nc.sync.dma_start(
                            out=st_g[:],
                            in_=st_d[:, g0 * WIN:(g0 + gT) * WIN])
                    goff = gt - g0
                    xsT_w = xsT_g[:, goff * TE:(goff + Tw) * TE]
                    xdT_w = xdT_g[:, goff * TE:(goff + Tw) * TE]
                    eaT_w = sp.tile([5, Tw * TE], f16, tag="eaT")
                    nc.sync.dma_start(
                        out=eaT_w[:], in_=eaT_d[:, gt * TE:(gt + Tw) * TE])
                    st_tiles[w] = st_g[:, goff * WIN:(goff + Tw) * WIN]

                    h_p = ph.tile([128, nb * 512], f32, tag="h")
                    hv = h_p[:].rearrange("p (b c) -> p b c", c=512)
                    for t in range(Tw):
                        b, k = t // 3, t % 3
                        osl = hv[:, b, k * D1:(k + 1) * D1]
                        nc.tensor.matmul(
                            osl, lhsT=xdT_w[:, t * TE:(t + 1) * TE],
                            rhs=W1DG[:], start=True, stop=False)
                        nc.tensor.matmul(
                            osl, lhsT=xsT_w[:, t * TE:(t + 1) * TE],
                            rhs=W1SG[:], start=False, stop=False)
                        nc.tensor.matmul(
                            osl, lhsT=eaT_w[:, t * TE:(t + 1) * TE],
                            rhs=W1EG[:], start=False, stop=True)

                    if SIM_MODE:
                        # pad tile slots are never consumed; CoreSim still
                        # requires initialized reads
                        for t in range(Tw, TP):
                            b, k = t // 3, t % 3
                            nc.vector.memset(
                                hv[:, b, k * D1:(k + 1) * D1], 0.0)
                    # h viewed [128, nb, 3, 136] (banks hold 3*136 + 104 pad)
                    hv4 = hv[:, :, 0:3 * D1].rearrange(
                        "p b (k c) -> p b k c", c=D1)
                    # mu: [128, nb, 3, 8] from h cols 128:136 (ACT copy)
                    mu_sb = wp.tile([128, nb_max * 24], f32, tag="mu")
                    muv = mu_sb[:, 0:TP * 8].rearrange(
                        "p (b k g) -> p b k g", k=3, g=8)
                    nc.scalar.copy(out=muv, in_=hv4[:, :, :, 128:136])

                    z_all = zz.tile([128, TP * TE], f16, tag="zall")
                    z_tiles[w] = z_all
                    # z1 = h - mu, per bank (ISA: STT inputs must lower to 3D)
                    zb = z_all[:].rearrange(
                        "p (b k g c) -> p b k g c", k=3, g=8, c=GSIZE)
                    for b in range(nb):
                        nc.vector.scalar_tensor_tensor(
                            out=zb[:, b],
                            in0=muv[:, b][:, :, :, None].to_broadcast(
                                [128, 3, 8, GSIZE]),
                            scalar=-1.0, op0=OP.mult, op1=OP.add,
                            in1=hv4[:, b, :, 0:128].rearrange(
                                "p k (g c) -> p k g c", c=GSIZE),
                        )
                    zq = zz.tile([128, Tw * TE], f16, tag="zsq")
                    zq_tiles[w] = zq
                    nc.scalar.activation(out=zq[:], in_=z_all[:, 0:Tw * TE],
                                         func=AF.Square)
                    # group sums: halving adds  16->8->4->2->1
                    zv = zq[:].rearrange("p (s g c) -> p s g c", g=8, c=GSIZE)
                    r8 = hh.tile([128, Tw * 64], f16, tag="r8")
                    r8v = r8[:].rearrange("p (s g c) -> p s g c", g=8, c=8)
                    nc.gpsimd.tensor_tensor(
                        out=r8v, in0=zv[:, :, :, 0:8], in1=zv[:, :, :, 8:16],
                        op=OP.add)
                    r4 = hh.tile([128, Tw * 32], f16, tag="r4")
                    r4v = r4[:].rearrange("p (s g c) -> p s g c", g=8, c=4)
                    nc.gpsimd.tensor_tensor(
                        out=r4v, in0=r8v[:, :, :, 0:4], in1=r8v[:, :, :, 4:8],
                        op=OP.add)
                    r2 = hh.tile([128, Tw * 16], f16, tag="r2")
                    r2v = r2[:].rearrange("p (s g c) -> p s g c", g=8, c=2)
                    nc.vector.tensor_tensor(
                        out=r2v, in0=r4v[:, :, :, 0:2], in1=r4v[:, :, :, 2:4],
                        op=OP.add)
                    nc.vector.tensor_tensor(
                        out=v_sb[:, voff[wi]:voff[wi + 1]].rearrange(
                            "p (s c) -> p s c", c=1),
                        in0=r2v[:, :, :, 0:1].rearrange("p s g c -> p (s g) c"),
                        in1=r2v[:, :, :, 1:2].rearrange("p s g c -> p (s g) c"),
                        op=OP.add)

                # ---- newton rsqrt batched over the superblock ----
                # vh = var/2 = v/32 + eps/2 ; quake seed vs vh needs the
                # magic shifted by 2^22 (vh = var/2 halves the exponent arg)
                vh = vv.tile([128, VW], f32, tag="vh")
                nc.vector.tensor_scalar(out=vh[:], in0=v_sb[:],
                                        scalar1=1.0 / (2 * GSIZE),
                                        scalar2=EPS / 2,
                                        op0=OP.mult, op1=OP.add)
                y = vv.tile([128, VW], f32, tag="y")
                nc.vector.tensor_scalar(
                    out=y[:].bitcast(i32), in0=vh[:].bitcast(i32), scalar1=1,
                    scalar2=None, op0=OP.logical_shift_right)
                nc.vector.tensor_scalar(
                    out=y[:].bitcast(i32), in0=y[:].bitcast(i32), scalar1=-1,
                    scalar2=0x5EF759DF, op0=OP.mult, op1=OP.add)
                a = vv.tile([128, VW], f32, tag="nta")
                nc.vector.tensor_tensor(out=a[:], in0=y[:], in1=y[:],
                                        op=OP.mult)
                nc.vector.tensor_tensor(out=a[:], in0=a[:], in1=vh[:],
                                        op=OP.mult)
                nc.vector.tensor_scalar(out=a[:], in0=a[:], scalar1=-1.0,
                                        scalar2=1.5, op0=OP.mult, op1=OP.add)
                inv16 = vv.tile([128, VW], f16, tag="inv16")
                nc.vector.tensor_tensor(out=inv16[:], in0=y[:], in1=a[:],
                                        op=OP.mult)

                state = (ws, voff, z_tiles, zq_tiles, st_tiles, inv16)
                if pending is not None:
                    run_phase2(pending)
                pending = state

            run_phase2(pending)

    nc.compile()
    return nc


def _prepare(x, edge_index, edge_attr, W1, b1, gn_gamma, gn_beta, W2, b2):
    x = np.ascontiguousarray(np.asarray(x, dtype=np.float32))
    W1 = np.asarray(W1, dtype=np.float32)
    b1 = np.asarray(b1, dtype=np.float32)
    W2 = np.asarray(W2, dtype=np.float32)
    b2 = np.asarray(b2, dtype=np.float32)
    gn_gamma = np.asarray(gn_gamma, dtype=np.float32)
    gn_beta = np.asarray(gn_beta, dtype=np.float32)

    T_ws, nwin, cap, node_counts, shards = _shard(np.asarray(edge_index),
                                                  edge_attr)
    nc = _build_program(T_ws, nwin)
    total_tiles = cap // TE

    G = np.zeros((OUT_DIM, N_GROUPS), dtype=np.float32)
    for g in range(N_GROUPS):
        G[g * GSIZE:(g + 1) * GSIZE, g] = 1.0 / GSIZE

    trivial = bool(np.all(gn_gamma == 1.0) and np.all(gn_beta == 0.0))
    assert trivial, "nontrivial GN affine not supported"

    W1d, W1s, W1e = W1[0:128], W1[128:256], W1[256:260]
    w1dg = np.concatenate([W1d, W1d @ G], axis=1).astype(np.float16)
    w1sg = np.concatenate([W1s, W1s @ G], axis=1).astype(np.float16)
    w1e_aug = np.concatenate([W1e, b1[None, :]], axis=0)
    w1eg = np.concatenate([w1e_aug, w1e_aug @ G], axis=1).astype(np.float16)

    x16 = x.astype(np.float16)
    ident = np.eye(128, dtype=np.float16)

    shared = {
        "w1dg": np.ascontiguousarray(w1dg),
        "w1sg": np.ascontiguousarray(w1sg),
        "w1eg": np.ascontiguousarray(w1eg),
        "w2": np.ascontiguousarray(W2).astype(np.float16),
        "ident": ident,
    }

    in_maps = []
    for c in range(N_CORES):
        srcc, dstc, dloc, eac = shards[c]
        xsT = np.ascontiguousarray(x16[srcc].T)
        xdT = np.ascontiguousarray(x16[dstc].T)
        eaT = np.empty((5, cap), dtype=np.float16)
        eaT[0:EDGE_DIM] = eac.T.astype(np.float16)
        eaT[EDGE_DIM] = 1.0
        import ml_dtypes
        st = np.zeros((TE, total_tiles * WIN), dtype=ml_dtypes.float8_e4m3)
        dl = dloc.reshape(total_tiles, TE)
        tt_idx, e_idx = np.nonzero(dl >= 0)
        st[e_idx, tt_idx * WIN + dl[tt_idx, e_idx]] = 1.0
        nrows = min(NPC, N_NODES - c * NPC)
        cnt = np.zeros((nwin * WIN,), dtype=np.float32)
        cnt[:nrows] = node_counts[c * NPC:c * NPC + nrows]
        invc = (1.0 / np.maximum(cnt, 1.0)).reshape(nwin, WIN).T
        b2i = ((cnt > 0).astype(np.float32)[:, None]
               * b2[None, :].astype(np.float32)).astype(np.float16)
        in_maps.append(dict(
            shared, xsT=xsT, xdT=xdT, eaT=eaT, st=st,
            invc=np.ascontiguousarray(invc),
            b2i=np.ascontiguousarray(b2i)))
    return nc, in_maps, nwin


def kernel(x, edge_index, edge_attr, W1, b1, gn_gamma, gn_beta, W2, b2):
    global LAST_EXEC_NS, LAST_RESULTS
    import os
    from concourse.bass_utils import run_bass_kernel_spmd

    nc, in_maps, nwin = _prepare(x, edge_index, edge_attr, W1, b1,
                                 gn_gamma, gn_beta, W2, b2)
    trace = bool(os.environ.get("BASS_TRACE"))
    res = run_bass_kernel_spmd(nc, in_maps, core_ids=list(range(N_CORES)),
                               trace=trace)
    LAST_EXEC_NS = res.exec_time_ns
    LAST_RESULTS = res

    out = np.empty((N_NODES, OUT_DIM), dtype=np.float32)
    for c in range(N_CORES):
        out[c * NPC:(c + 1) * NPC] = res.results[c]["out"][:NPC]
    return out
